# revision 11
# baseline (speedup 1.0000x reference)
"""Trainium2 Bass kernel for MixedIntQuantizedLinear.

Computation (see reference):
  W_dq[o,i] = W_int[o,i] * (scale_i32[o, i//64] / 2^24)
  per-token: amax_t = clip(max|x_t|, 1e-8); s_t = amax_t/127
             q_t = round(x_t / s_t)  (|q| <= 127, round-to-nearest-even)
  y[t,o] = s_t * sum_i q_t[i] * W_dq[o,i] + bias[o]

Sharding over 8 NeuronCores: 2 token-groups (batch halves) x 4
out-feature groups of 1024.  Each core computes y_core [4096, 1024].

Per-core kernel strategy (v4):
  - W ships as int8 (lossless: values in [-127,127]) to quarter DMA
    bytes.  Device: ScalarE int8->fp32, DVE multiply by broadcast block
    scales -> bf16, XBAR DMA-transpose into resident WT tensors.
    W is processed in quarter-stripes, k-major, into 8 separate WT
    tensors [128, 8, 512] so the first matmuls start after ~1/32 of
    W-prep instead of all of it.  PE does matmuls only.
  - x tiles [128 tok, 4096]: DVE absmax-reduce; quantize via the fp32
    magic-number trick (x*inv + 1.5*2^23 rounds to int with plain fp32
    RNE); ScalarE subtracts the magic and emits exact-integer bf16;
    one XBAR DMA-transpose produces qT [128, 32, 128].
  - 32 accumulating bf16 matmuls per PSUM tile [128 tok, 512 out];
    epilogue: ScalarE copy with per-partition scale s_t, DVE adds the
    (PE-broadcast) bias row, DMA out.
"""

import os
import sys

sys.path.insert(0, "/opt/trn_rl_repo")

import numpy as np

import concourse.bass as bass
import concourse.tile as tile
from concourse import bacc, mybir
from concourse.bass_utils import run_bass_kernel_spmd

P = 128
IN_F = 4096
OUT_F = 4096
TOKENS = 8192          # 4 * 2048
N_CORES = 8
TG = 2                 # token groups
OG = 4                 # out-feature groups
T_CORE = TOKENS // TG  # 4096 tokens per core
O_CORE = OUT_F // OG   # 1024 out features per core
KT = IN_F // P         # 32 contraction tiles
TT = T_CORE // P       # 32 token tiles
OC = O_CORE // 512     # 2 psum chunks of 512
BLOCK = 64
MAGIC = 12582912.0     # 1.5 * 2^23: fp32 round-to-int magic constant
INV_SCALE_SHIFT = 1.0 / (1 << 24)

QF = IN_F // 4         # 1024: quarter-stripe width
QB = QF // BLOCK       # 16 blocks per quarter
QK = KT // 4           # 8 k-tiles per quarter

F32 = mybir.dt.float32
BF16 = mybir.dt.bfloat16
I32 = mybir.dt.int32
I8 = mybir.dt.int8
ACT_COPY = mybir.ActivationFunctionType.Copy


def build_kernel(reps=1):
    nc = bacc.Bacc(None, target_bir_lowering=False, debug=False)

    x_d = nc.dram_tensor("x", [T_CORE, IN_F], F32, kind="ExternalInput")
    w_d = nc.dram_tensor("w", [O_CORE, IN_F], I8, kind="ExternalInput")
    s_d = nc.dram_tensor("s", [O_CORE, BLOCK], I32, kind="ExternalInput")
    b_d = nc.dram_tensor("b", [1, O_CORE], F32, kind="ExternalInput")
    y_d = nc.dram_tensor("y", [T_CORE, O_CORE], F32, kind="ExternalOutput")

    with tile.TileContext(nc) as tc:
        with (
            tc.tile_pool(name="const", bufs=1) as const_pool,
            tc.tile_pool(name="wt", bufs=1) as wt_pool,
            tc.tile_pool(name="psum_y", bufs=6, space="PSUM") as psum_y,
            tc.tile_pool(name="psum_misc", bufs=2, space="PSUM") as psum_misc,
        ):
            # ---- bias broadcast row -> [128, O_CORE] via K=1 matmul ----
            ones_k1 = const_pool.tile([1, P], F32)
            nc.vector.memset(ones_k1[:], 1.0)
            bias_sb = const_pool.tile([1, O_CORE], F32)
            nc.sync.dma_start(bias_sb[:], b_d[:])
            bias_bcast = const_pool.tile([P, O_CORE], F32)
            for oc in range(OC):
                pb = psum_misc.tile([P, 512], F32, tag="ptr")
                nc.tensor.matmul(
                    pb[:], ones_k1[:], bias_sb[:, oc * 512:(oc + 1) * 512],
                    start=True, stop=True,
                )
                nc.scalar.copy(bias_bcast[:, oc * 512:(oc + 1) * 512], pb[:])

            # ---- block scales -> fp32 [128, 8, 64] ----
            n_str = O_CORE // P  # 8 weight stripes
            sc_i32 = const_pool.tile([P, n_str, BLOCK], I32)
            nc.sync.dma_start(
                sc_i32[:], s_d.rearrange("(s p) b -> p s b", p=P)
            )
            sc_f32 = const_pool.tile([P, n_str, BLOCK], F32)
            nc.vector.tensor_copy(sc_f32[:], sc_i32[:])
            nc.vector.tensor_scalar_mul(sc_f32[:], sc_f32[:], INV_SCALE_SHIFT)

            # WT tensors, one per (oc chunk, k quarter): [128, QK, 512]
            wtq = [[None] * 4 for _ in range(OC)]
            for oc in range(OC):
                for kq in range(4):
                    w = wt_pool.tile([P, QK, 512], BF16, name=f"wt{oc}_{kq}",
                                     tag=f"wt{oc}_{kq}")
                    wtq[oc][kq] = w

            with (
                tc.tile_pool(name="wprep", bufs=3) as wprep,
                tc.tile_pool(name="xin", bufs=3) as xin_pool,
                tc.tile_pool(name="small", bufs=6) as small_pool,
                tc.tile_pool(name="qb", bufs=3) as qb_pool,
                tc.tile_pool(name="qt", bufs=3) as qt_pool,
                tc.tile_pool(name="orow", bufs=3) as orow_pool,
            ):
                # prefetch first x tiles so quant pipeline overlaps W-prep
                PREFETCH = 2
                xts = {}
                for tt in range(PREFETCH):
                    xt = xin_pool.tile([P, IN_F], F32, tag="xt")
                    nc.sync.dma_start(xt[:], x_d[tt * P:(tt + 1) * P, :])
                    xts[tt] = xt

                # ---- W: int8 -> fp32 -> dequant bf16 -> XBAR into WT ----
                # k-quarter-major so wtq[0][0] completes first.
                for kq in range(4):
                    for s in range(n_str):
                        w_i8 = wprep.tile([P, QF], I8, tag="w_i8")
                        nc.sync.dma_start(
                            w_i8[:],
                            w_d[s * P:(s + 1) * P, kq * QF:(kq + 1) * QF])
                        w_f32 = wprep.tile([P, QF], F32, tag="w_f32")
                        nc.scalar.copy(w_f32[:], w_i8[:])
                        w_bf = wprep.tile([P, QF], BF16, tag="w_bf")
                        nc.vector.tensor_tensor(
                            w_bf.rearrange("p (nb j) -> p nb j", j=BLOCK),
                            w_f32.rearrange("p (nb j) -> p nb j", j=BLOCK),
                            sc_f32[:, s, kq * QB:(kq + 1) * QB, None]
                            .to_broadcast((P, QB, BLOCK)),
                            mybir.AluOpType.mult,
                        )
                        nc.sync.dma_start_transpose(
                            wtq[s // 4][kq][:, :, (s % 4) * P:(s % 4 + 1) * P],
                            w_bf[:],
                        )

                # ---- main token loop ----
                for tt_rep in range(TT * reps):
                    tt = tt_rep % TT
                    if tt in xts and tt_rep < TT:
                        xt = xts.pop(tt)
                    else:
                        xt = xin_pool.tile([P, IN_F], F32, tag="xt")
                        nc.sync.dma_start(xt[:], x_d[tt * P:(tt + 1) * P, :])

                    amax = small_pool.tile([P, 1], F32, tag="amax")
                    nc.vector.tensor_reduce(
                        amax[:], xt[:], axis=mybir.AxisListType.X,
                        op=mybir.AluOpType.max, apply_absolute_value=True,
                    )
                    nc.vector.tensor_scalar_max(amax[:], amax[:], 1e-8)
                    s_t = small_pool.tile([P, 1], F32, tag="s_t")
                    nc.vector.tensor_scalar_mul(s_t[:], amax[:], 1.0 / 127.0)
                    inv = small_pool.tile([P, 1], F32, tag="inv")
                    nc.vector.reciprocal(inv[:], s_t[:])

                    # x <- x * inv + MAGIC  (fp32; integer part = q + MAGIC)
                    nc.vector.tensor_scalar(
                        xt[:], xt[:], inv[:], MAGIC,
                        op0=mybir.AluOpType.mult, op1=mybir.AluOpType.add,
                    )
                    # q (exact small ints) in bf16, via ScalarE
                    qb = qb_pool.tile([P, IN_F], BF16, tag="qb")
                    nc.scalar.activation(qb[:], xt[:], ACT_COPY, bias=-MAGIC)

                    # XBAR transpose -> qT [128(i), KT, 128(t)]
                    qt = qt_pool.tile([P, KT, P], BF16, tag="qt")
                    nc.sync.dma_start_transpose(qt[:], qb[:])

                    orow = orow_pool.tile([P, O_CORE], F32, tag="orow")
                    for oc in range(OC):
                        py = psum_y.tile([P, 512], F32, tag="py")
                        for k in range(KT):
                            nc.tensor.matmul(
                                py[:], qt[:, k, :],
                                wtq[oc][k // QK][:, k % QK, :],
                                start=(k == 0), stop=(k == KT - 1),
                            )
                        nc.scalar.activation(
                            orow[:, oc * 512:(oc + 1) * 512], py[:],
                            ACT_COPY, scale=s_t[:],
                        )
                        nc.vector.tensor_tensor(
                            orow[:, oc * 512:(oc + 1) * 512],
                            orow[:, oc * 512:(oc + 1) * 512],
                            bias_bcast[:, oc * 512:(oc + 1) * 512],
                            mybir.AluOpType.add,
                        )
                    nc.sync.dma_start(y_d[tt * P:(tt + 1) * P, :], orow[:])

    nc.compile()
    return nc


_NC_CACHE = None


def _get_nc():
    global _NC_CACHE
    if _NC_CACHE is None:
        _NC_CACHE = build_kernel()
    return _NC_CACHE


def kernel(x, W_int, scale_i32, bias, _trace=False, _tmpdir=None):
    nc = _get_nc()
    x2 = np.ascontiguousarray(x, dtype=np.float32).reshape(TOKENS, IN_F)
    W_i8 = np.asarray(W_int).astype(np.int8)          # lossless: [-127,127]
    scale_i32 = np.asarray(scale_i32, dtype=np.int32)
    bias2 = np.asarray(bias, dtype=np.float32).reshape(1, OUT_F)

    in_maps = []
    for c in range(N_CORES):
        tg, og = c // OG, c % OG
        in_maps.append({
            "x": np.ascontiguousarray(x2[tg * T_CORE:(tg + 1) * T_CORE]),
            "w": np.ascontiguousarray(W_i8[og * O_CORE:(og + 1) * O_CORE]),
            "s": np.ascontiguousarray(
                scale_i32[og * O_CORE:(og + 1) * O_CORE]),
            "b": np.ascontiguousarray(bias2[:, og * O_CORE:(og + 1) * O_CORE]),
        })

    res = run_bass_kernel_spmd(
        nc, in_maps, core_ids=list(range(N_CORES)),
        trace=_trace, tmpdir=_tmpdir,
    )
    y = np.empty((TOKENS, OUT_F), dtype=np.float32)
    for c in range(N_CORES):
        tg, og = c // OG, c % OG
        y[tg * T_CORE:(tg + 1) * T_CORE, og * O_CORE:(og + 1) * O_CORE] = \
            res.results[c]["y"]
    out = y.reshape(4, 2048, OUT_F)
    if _trace:
        return out, res
    return out


# revision 15
# speedup vs baseline: 150.6760x; 150.6760x over previous
"""Trainium2 Bass kernel for MixedIntQuantizedLinear.

Computation (see reference):
  W_dq[o,i] = W_int[o,i] * (scale_i32[o, i//64] / 2^24)
  per-token: amax_t = clip(max|x_t|, 1e-8); s_t = amax_t/127
             q_t = round(x_t / s_t)  (|q| <= 127, round-to-nearest-even)
  y[t,o] = s_t * sum_i q_t[i] * W_dq[o,i] + bias[o]

Sharding over 8 NeuronCores: 2 token-groups (batch halves) x 4
out-feature groups of 1024.  Each core computes y_core [4096, 1024].

Per-core kernel strategy (v4):
  - W ships as int8 (lossless: values in [-127,127]) to quarter DMA
    bytes.  Device: ScalarE int8->fp32, DVE multiply by broadcast block
    scales -> bf16, XBAR DMA-transpose into resident WT tensors.
    W is processed in quarter-stripes, k-major, into 8 separate WT
    tensors [128, 8, 512] so the first matmuls start after ~1/32 of
    W-prep instead of all of it.  PE does matmuls only.
  - x tiles [128 tok, 4096]: DVE absmax-reduce; quantize via the fp32
    magic-number trick (x*inv + 1.5*2^23 rounds to int with plain fp32
    RNE); ScalarE subtracts the magic and emits exact-integer bf16;
    one XBAR DMA-transpose produces qT [128, 32, 128].
  - 32 accumulating bf16 matmuls per PSUM tile [128 tok, 512 out];
    epilogue: ScalarE copy with per-partition scale s_t, DVE adds the
    (PE-broadcast) bias row, DMA out.
"""

import os
import sys

sys.path.insert(0, "/opt/trn_rl_repo")

import numpy as np

import concourse.bass as bass
import concourse.tile as tile
from concourse import bacc, mybir
from concourse.bass_utils import run_bass_kernel_spmd

P = 128
IN_F = 4096
OUT_F = 4096
TOKENS = 8192          # 4 * 2048
N_CORES = 8
TG = 2                 # token groups
OG = 4                 # out-feature groups
T_CORE = TOKENS // TG  # 4096 tokens per core
O_CORE = OUT_F // OG   # 1024 out features per core
KT = IN_F // P         # 32 contraction tiles
TT = T_CORE // P       # 32 token tiles
OC = O_CORE // 512     # 2 psum chunks of 512
BLOCK = 64
MAGIC = 12582912.0     # 1.5 * 2^23: fp32 round-to-int magic constant
INV_SCALE_SHIFT = 1.0 / (1 << 24)

QF = IN_F // 4         # 1024: quarter-stripe width
QB = QF // BLOCK       # 16 blocks per quarter
QK = KT // 4           # 8 k-tiles per quarter

F32 = mybir.dt.float32
BF16 = mybir.dt.bfloat16
I32 = mybir.dt.int32
I8 = mybir.dt.int8
ACT_COPY = mybir.ActivationFunctionType.Copy


def build_kernel(reps=1, dyn_loop_n=None):
    """dyn_loop_n: if set, wrap the token loop in a hardware For_i loop
    with that trip count (benchmarking only)."""
    nc = bacc.Bacc(None, target_bir_lowering=False, debug=False)

    x_d = nc.dram_tensor("x", [T_CORE, IN_F], F32, kind="ExternalInput")
    w_d = nc.dram_tensor("w", [O_CORE, IN_F], I8, kind="ExternalInput")
    s_d = nc.dram_tensor("s", [O_CORE, BLOCK], I32, kind="ExternalInput")
    b_d = nc.dram_tensor("b", [1, O_CORE], F32, kind="ExternalInput")
    y_d = nc.dram_tensor("y", [T_CORE, O_CORE], F32, kind="ExternalOutput")

    with tile.TileContext(nc) as tc:
        with (
            tc.tile_pool(name="const", bufs=1) as const_pool,
            tc.tile_pool(name="wt", bufs=1) as wt_pool,
            tc.tile_pool(name="psum_y", bufs=6, space="PSUM") as psum_y,
            tc.tile_pool(name="psum_misc", bufs=2, space="PSUM") as psum_misc,
        ):
            # ---- bias broadcast row -> [128, O_CORE] via K=1 matmul ----
            ones_k1 = const_pool.tile([1, P], F32)
            nc.vector.memset(ones_k1[:], 1.0)
            bias_sb = const_pool.tile([1, O_CORE], F32)
            nc.sync.dma_start(bias_sb[:], b_d[:])
            bias_bcast = const_pool.tile([P, O_CORE], F32)
            for oc in range(OC):
                pb = psum_misc.tile([P, 512], F32, tag="ptr")
                nc.tensor.matmul(
                    pb[:], ones_k1[:], bias_sb[:, oc * 512:(oc + 1) * 512],
                    start=True, stop=True,
                )
                nc.scalar.copy(bias_bcast[:, oc * 512:(oc + 1) * 512], pb[:])

            # ---- block scales -> fp32 [128, 8, 64] ----
            n_str = O_CORE // P  # 8 weight stripes
            sc_i32 = const_pool.tile([P, n_str, BLOCK], I32)
            nc.sync.dma_start(
                sc_i32[:], s_d.rearrange("(s p) b -> p s b", p=P)
            )
            sc_f32 = const_pool.tile([P, n_str, BLOCK], F32)
            nc.vector.tensor_copy(sc_f32[:], sc_i32[:])
            nc.vector.tensor_scalar_mul(sc_f32[:], sc_f32[:], INV_SCALE_SHIFT)

            # WT tensors, one per (oc chunk, k quarter): [128, QK, 512]
            wtq = [[None] * 4 for _ in range(OC)]
            for oc in range(OC):
                for kq in range(4):
                    w = wt_pool.tile([P, QK, 512], BF16, name=f"wt{oc}_{kq}",
                                     tag=f"wt{oc}_{kq}")
                    wtq[oc][kq] = w

            with (
                tc.tile_pool(name="wprep", bufs=3) as wprep,
                tc.tile_pool(name="xin", bufs=3) as xin_pool,
                tc.tile_pool(name="small", bufs=6) as small_pool,
                tc.tile_pool(name="qb", bufs=3) as qb_pool,
                tc.tile_pool(name="qt", bufs=3) as qt_pool,
                tc.tile_pool(name="orow", bufs=3) as orow_pool,
            ):
                # prefetch first x tiles so quant pipeline overlaps W-prep
                PREFETCH = 0 if dyn_loop_n is not None else 2
                xts = {}
                for tt in range(PREFETCH):
                    xt = xin_pool.tile([P, IN_F], F32, tag="xt")
                    nc.sync.dma_start(xt[:], x_d[tt * P:(tt + 1) * P, :])
                    xts[tt] = xt

                # ---- W: int8 -> fp32 -> dequant bf16 -> XBAR into WT ----
                # k-quarter-major so wtq[0][0] completes first.
                for kq in range(4):
                    for s in range(n_str):
                        w_i8 = wprep.tile([P, QF], I8, tag="w_i8")
                        nc.sync.dma_start(
                            w_i8[:],
                            w_d[s * P:(s + 1) * P, kq * QF:(kq + 1) * QF])
                        w_f32 = wprep.tile([P, QF], F32, tag="w_f32")
                        nc.scalar.copy(w_f32[:], w_i8[:])
                        w_bf = wprep.tile([P, QF], BF16, tag="w_bf")
                        nc.vector.tensor_tensor(
                            w_bf.rearrange("p (nb j) -> p nb j", j=BLOCK),
                            w_f32.rearrange("p (nb j) -> p nb j", j=BLOCK),
                            sc_f32[:, s, kq * QB:(kq + 1) * QB, None]
                            .to_broadcast((P, QB, BLOCK)),
                            mybir.AluOpType.mult,
                        )
                        nc.sync.dma_start_transpose(
                            wtq[s // 4][kq][:, :, (s % 4) * P:(s % 4 + 1) * P],
                            w_bf[:],
                        )

                # ---- main token loop ----
                def token_loop(first_pass):
                    for tt in range(TT):
                        _token_tile(tt, first_pass)

                def _token_tile(tt, first_pass):
                    if tt in xts and first_pass:
                        xt = xts.pop(tt)
                    else:
                        xt = xin_pool.tile([P, IN_F], F32, tag="xt")
                        nc.sync.dma_start(xt[:], x_d[tt * P:(tt + 1) * P, :])

                    amax = small_pool.tile([P, 1], F32, tag="amax")
                    nc.vector.tensor_reduce(
                        amax[:], xt[:], axis=mybir.AxisListType.X,
                        op=mybir.AluOpType.max, apply_absolute_value=True,
                    )
                    nc.vector.tensor_scalar_max(amax[:], amax[:], 1e-8)
                    s_t = small_pool.tile([P, 1], F32, tag="s_t")
                    nc.vector.tensor_scalar_mul(s_t[:], amax[:], 1.0 / 127.0)
                    inv = small_pool.tile([P, 1], F32, tag="inv")
                    nc.vector.reciprocal(inv[:], s_t[:])

                    # x <- x * inv + MAGIC  (fp32; integer part = q + MAGIC)
                    nc.vector.tensor_scalar(
                        xt[:], xt[:], inv[:], MAGIC,
                        op0=mybir.AluOpType.mult, op1=mybir.AluOpType.add,
                    )
                    # q (exact small ints) in bf16, via ScalarE
                    qb = qb_pool.tile([P, IN_F], BF16, tag="qb")
                    nc.scalar.activation(qb[:], xt[:], ACT_COPY, bias=-MAGIC)

                    # XBAR transpose -> qT [128(i), KT, 128(t)]
                    qt = qt_pool.tile([P, KT, P], BF16, tag="qt")
                    nc.sync.dma_start_transpose(qt[:], qb[:])

                    orow = orow_pool.tile([P, O_CORE], F32, tag="orow")
                    for oc in range(OC):
                        py = psum_y.tile([P, 512], F32, tag="py")
                        for k in range(KT):
                            nc.tensor.matmul(
                                py[:], qt[:, k, :],
                                wtq[oc][k // QK][:, k % QK, :],
                                start=(k == 0), stop=(k == KT - 1),
                            )
                        nc.scalar.activation(
                            orow[:, oc * 512:(oc + 1) * 512], py[:],
                            ACT_COPY, scale=s_t[:],
                        )
                        nc.vector.tensor_tensor(
                            orow[:, oc * 512:(oc + 1) * 512],
                            orow[:, oc * 512:(oc + 1) * 512],
                            bias_bcast[:, oc * 512:(oc + 1) * 512],
                            mybir.AluOpType.add,
                        )
                    nc.sync.dma_start(y_d[tt * P:(tt + 1) * P, :], orow[:])

                if dyn_loop_n is not None:
                    with tc.For_i(0, dyn_loop_n, 1):
                        token_loop(first_pass=False)
                else:
                    for rep in range(reps):
                        token_loop(first_pass=(rep == 0))

    nc.compile()
    return nc


_NC_CACHE = None


def _get_nc():
    global _NC_CACHE
    if _NC_CACHE is None:
        _NC_CACHE = build_kernel()
    return _NC_CACHE


def kernel(x, W_int, scale_i32, bias, _trace=False, _tmpdir=None):
    nc = _get_nc()
    x2 = np.ascontiguousarray(x, dtype=np.float32).reshape(TOKENS, IN_F)
    W_i8 = np.asarray(W_int).astype(np.int8)          # lossless: [-127,127]
    scale_i32 = np.asarray(scale_i32, dtype=np.int32)
    bias2 = np.asarray(bias, dtype=np.float32).reshape(1, OUT_F)

    in_maps = []
    for c in range(N_CORES):
        tg, og = c // OG, c % OG
        in_maps.append({
            "x": np.ascontiguousarray(x2[tg * T_CORE:(tg + 1) * T_CORE]),
            "w": np.ascontiguousarray(W_i8[og * O_CORE:(og + 1) * O_CORE]),
            "s": np.ascontiguousarray(
                scale_i32[og * O_CORE:(og + 1) * O_CORE]),
            "b": np.ascontiguousarray(bias2[:, og * O_CORE:(og + 1) * O_CORE]),
        })

    res = run_bass_kernel_spmd(
        nc, in_maps, core_ids=list(range(N_CORES)),
        trace=_trace, tmpdir=_tmpdir,
    )
    y = np.empty((TOKENS, OUT_F), dtype=np.float32)
    for c in range(N_CORES):
        tg, og = c // OG, c % OG
        y[tg * T_CORE:(tg + 1) * T_CORE, og * O_CORE:(og + 1) * O_CORE] = \
            res.results[c]["y"]
    out = y.reshape(4, 2048, OUT_F)
    if _trace:
        return out, res
    return out
